# revision 1
# baseline (speedup 1.0000x reference)
# Trainium2 Bass kernel: depthwise 3D Gaussian low-pass filter (5x5x5, separable)
# image [4, 64, 64, 64, 32] (n, x, y, z, c) -> same-shape output, stride 1, pad 2.
#
# Sharding: 8 cores = (n, z-half). Each core owns n = k//2, z in [32*(k%2), +32)
# and loads a z-halo of 2 on each side (zero-padded at volume edges).
#
# Per core, two PE stages on [128, free] tiles with partitions = (y-parity a, x):
#   stage 1: conv over x and y in one PSUM accumulation group: out_block b =
#            sum_d W_d.T @ in_block(b+d), d in {-1,0,1}, where W_d[128,128]
#            = Toeplitz_x (5-tap) x y-parity band. 3 matmuls per output block.
#   stage 2: conv over z as 5 shifted matmuls with scaled-identity weights
#            (z is in the free dim, stride C; shift = j*C elements).
# All matmuls in float32r (1 cycle/row at N>=256; ~1.6e-4 scale-relative error).
import numpy as np

_SIGMA = 0.5 * (2.0 ** 2 - 1) ** 0.5  # scale = 2.0
_KS = 5
_NC = 8
_X, _Y, _Z, _C, _NB = 64, 64, 64, 32, 4
_ZH = _Z // 2          # z extent per core (32)
_ZP = _ZH + 4          # with halo (36)
_FIN = _ZP * _C        # 1152 free elements in
_FOUT = _ZH * _C       # 1024 free elements out
_NBLK = _Y // 2        # 32 y-blocks

_CACHE = {}


def _wn():
    r = np.arange(_KS, dtype=np.float64) - _KS // 2
    w = np.exp(-(r ** 2) / (2 * _SIGMA ** 2))
    return w / w.sum()


def _build_consts():
    wn = _wn()
    Bx = np.zeros((64, 64))
    for x in range(64):
        for xp in range(max(0, x - 2), min(64, x + 3)):
            Bx[x, xp] = wn[xp - x + 2]
    wmat = np.zeros((3, 128, 128))
    for di, d in enumerate((-1, 0, 1)):
        for a in range(2):
            for a2 in range(2):
                idx = 2 * d + a - a2 + 2
                if 0 <= idx < _KS:
                    wmat[di, a * 64:(a + 1) * 64, a2 * 64:(a2 + 1) * 64] = Bx * wn[idx]
    zmat = np.zeros((_KS, 128, 128))
    for j in range(_KS):
        zmat[j] = np.eye(128) * wn[j]
    return wmat.astype(np.float32), zmat.astype(np.float32)


def _build_nc():
    import concourse.bacc as bacc
    import concourse.mybir as mybir
    import concourse.tile as tile

    f32 = mybir.dt.float32
    f32r = mybir.dt.float32r

    nc = bacc.Bacc("TRN2", target_bir_lowering=False, debug=False,
                   num_devices=_NC)
    xin = nc.dram_tensor("xin", [_X, _Y, _ZP, _C], f32r, kind="ExternalInput")
    wm = nc.dram_tensor("wm", [3 * 128, 128], f32r, kind="ExternalInput")
    zm = nc.dram_tensor("zm", [_KS * 128, 128], f32r, kind="ExternalInput")
    yout = nc.dram_tensor("yout", [_X, _Y, _ZH, _C], f32, kind="ExternalOutput")

    # [x, y, z, c] -> [b, a, x, (z c)]; partition p = a*64 + x needs two DMAs
    # per tile (a=0 -> partitions 0:64, a=1 -> 64:128) since (a x) strides
    # can't fuse into one AP dim.
    xin_v = xin.ap().rearrange("x (b a) z c -> b a x (z c)", a=2)
    yout_v = yout.ap().rearrange("x (b a) z c -> b a x (z c)", a=2)

    with tile.TileContext(nc) as tc:
        with (
            tc.tile_pool(name="consts", bufs=1) as cpool,
            tc.tile_pool(name="xin", bufs=8) as xpool,
            tc.tile_pool(name="s1", bufs=4) as s1pool,
            tc.tile_pool(name="out", bufs=4) as opool,
            tc.tile_pool(name="tmp", bufs=4) as tpool,
            tc.tile_pool(name="psum1", bufs=4, space="PSUM") as p1pool,
            tc.tile_pool(name="psum2", bufs=4, space="PSUM") as p2pool,
        ):
            wnf = _wn()
            Wt = []
            for di in range(3):
                t = cpool.tile([128, 128], f32r, tag=f"w{di}")
                nc.sync.dma_start(out=t[:], in_=wm.ap()[di * 128:(di + 1) * 128, :])
                Wt.append(t)
            Zt = []
            for j in range(_KS):
                t = cpool.tile([128, 128], f32r, tag=f"z{j}")
                nc.sync.dma_start(out=t[:], in_=zm.ap()[j * 128:(j + 1) * 128, :])
                Zt.append(t)

            xt = {}

            def load(b):
                if 0 <= b < _NBLK and b not in xt:
                    t = xpool.tile([128, _FIN], f32r, tag="xt")
                    nc.gpsimd.dma_start(out=t[0:64, :], in_=xin_v[b, 0])
                    nc.gpsimd.dma_start(out=t[64:128, :], in_=xin_v[b, 1])
                    xt[b] = t

            load(0)
            load(1)
            for b in range(_NBLK):
                load(b + 2)
                # stage 1: conv_xy -> S1 [(a,x), (z36, c)]
                s1 = s1pool.tile([128, _FIN], f32r, tag="s1")
                for ch in range(3):
                    lo = ch * 384
                    p1 = p1pool.tile([128, 384], f32, tag="p1")
                    ds = [d for d in (-1, 0, 1) if 0 <= b + d < _NBLK]
                    for i, d in enumerate(ds):
                        nc.tensor.matmul(
                            p1[:], Wt[d + 1][:], xt[b + d][:, lo:lo + 384],
                            start=(i == 0), stop=(i == len(ds) - 1))
                    if ch == 1:
                        nc.vector.tensor_copy(s1[:, lo:lo + 384], p1[:])
                    else:
                        nc.scalar.copy(s1[:, lo:lo + 384], p1[:])
                # stage 2: conv_z -> O [(a,x), (z32, c)].  PE does the 5-tap
                # z-conv as shifted scaled-identity matmuls; on odd blocks the
                # second output chunk runs on DVE instead (STT fma chain) to
                # balance PE against the otherwise-idle vector engine.
                o = opool.tile([128, _FOUT], f32, tag="o")
                s1f = s1[:].bitcast(f32)
                dve_chunks = (1,) if (b % 2) else ()
                for ch in range(2):
                    lo = ch * 512
                    if ch in dve_chunks:
                        t0 = tpool.tile([128, 512], f32, tag="t0")
                        t1 = tpool.tile([128, 512], f32, tag="t1")
                        acc, nxt = t0, t1
                        nc.vector.tensor_scalar_mul(
                            acc[:], s1f[:, lo:lo + 512], float(wnf[0]))
                        for j in range(1, _KS):
                            dst = o[:, lo:lo + 512] if j == _KS - 1 else nxt[:]
                            nc.vector.scalar_tensor_tensor(
                                dst, s1f[:, lo + j * _C: lo + j * _C + 512],
                                float(wnf[j]), acc[:],
                                mybir.AluOpType.mult, mybir.AluOpType.add)
                            acc, nxt = nxt, acc
                    else:
                        p2 = p2pool.tile([128, 512], f32, tag="p2")
                        for j in range(_KS):
                            nc.tensor.matmul(
                                p2[:], Zt[j][:], s1[:, lo + j * _C: lo + j * _C + 512],
                                start=(j == 0), stop=(j == _KS - 1))
                        nc.scalar.copy(o[:, lo:lo + 512], p2[:])
                st_eng = nc.sync if (b % 2) else nc.scalar
                st_eng.dma_start(out=yout_v[b, 0], in_=o[0:64, :])
                st_eng.dma_start(out=yout_v[b, 1], in_=o[64:128, :])
    nc.compile()
    return nc


def kernel(image, kernel, _trace=False):
    from concourse.bass_utils import run_bass_kernel_spmd

    image = np.ascontiguousarray(np.asarray(image), dtype=np.float32)
    if "nc" not in _CACHE:
        _CACHE["nc"] = _build_nc()
        _CACHE["consts"] = _build_consts()
    nc = _CACHE["nc"]
    wmat, zmat = _CACHE["consts"]
    wm = wmat.reshape(3 * 128, 128)
    zm = zmat.reshape(_KS * 128, 128)

    in_maps = []
    for k in range(_NC):
        n, h = k // 2, k % 2
        zlo = h * _ZH - 2
        xin = np.zeros((_X, _Y, _ZP, _C), np.float32)
        s0, s1 = max(0, zlo), min(_Z, zlo + _ZP)
        xin[:, :, s0 - zlo: s1 - zlo, :] = image[n, :, :, s0:s1, :]
        in_maps.append({"xin": xin, "wm": wm, "zm": zm})

    res = run_bass_kernel_spmd(nc, in_maps, list(range(_NC)), trace=_trace)
    out = np.empty((_NB, _X, _Y, _Z, _C), np.float32)
    for k in range(_NC):
        n, h = k // 2, k % 2
        out[n, :, :, h * _ZH:(h + 1) * _ZH, :] = res.results[k]["yout"]
    if _trace:
        return out, res
    return out

